# revision 8
# baseline (speedup 1.0000x reference)
"""MoE top-2 routing kernel for 8 Trainium2 NeuronCores (Bass/Tile).

Strategy: data-parallel over tokens (4096/core). Per core:
  gate matmul in true fp32 (4-pass PE) on x^T assembled from host-split
  bf16 hi/lo halves (loaded transposed via dma_gather transpose);
  top-2 on logits via DVE compare/reduce; softmax probs via ACT exp;
  probabilities folded into x (per-partition scale) before dispatch;
  per-(expert, round) token lists built by masked sparse_gather;
  dispatch via SBUF-source dma_gather(transpose) -> bf16 expert matmuls;
  combine via dma_scatter_add into two bf16 staging planes (round A/B,
  race-free), final fused add -> fp32 output rows. Load-balance loss
  partial sums (per-expert prob column sums) computed via PE and reduced
  on host along with the trivial final scalar formula.
"""
import numpy as np
import ml_dtypes

NOISE_STD = 0.1
E = 32          # experts
TOPK = 2
D = 256         # output dim per expert
J = 256         # input dim
NCORE = 8
NT = 4096       # tokens per core
G = NT // 128   # 32 token tiles per core
C = 256         # capacity per (expert, round); counts ~128 +- 12
FW = C // 16
BIG = 1.0e4

_CACHE = {}


def _build(has_gate_b, has_eb):
    import concourse.bass as bass
    import concourse.bacc as bacc
    import concourse.tile as tile
    import concourse.mybir as mybir

    f32 = mybir.dt.float32
    bf16 = mybir.dt.bfloat16
    i16 = mybir.dt.int16
    u32 = mybir.dt.uint32
    AF = mybir.ActivationFunctionType
    OP = mybir.AluOpType
    AX = mybir.AxisListType

    nc = bacc.Bacc("TRN2", target_bir_lowering=False, debug=False, num_devices=NCORE)

    xhi_d = nc.declare_dram_parameter("xhi", [NT, J], bf16, isOutput=False)
    xlo_d = nc.declare_dram_parameter("xlo", [NT, J], bf16, isOutput=False)
    noi_d = nc.declare_dram_parameter("noi", [NT, E], f32, isOutput=False)
    gwT_d = nc.declare_dram_parameter("gwT", [J, E], f32, isOutput=False)
    wT_d = nc.declare_dram_parameter("wT", [E, 2, 128, D], bf16, isOutput=False)
    i128_d = nc.declare_dram_parameter("i128", [128, 128], f32, isOutput=False)
    em64_d = nc.declare_dram_parameter("em64", [128, E], f32, isOutput=False)
    iow_d = nc.declare_dram_parameter("iow", [16, G, 8], f32, isOutput=False)
    ioid_d = nc.declare_dram_parameter("ioid", [128, NT // 16], i16, isOutput=False)
    if has_gate_b:
        gb_d = nc.declare_dram_parameter("gb", [1, E], f32, isOutput=False)
    if has_eb:
        eb_d = nc.declare_dram_parameter("eb", [E, D], bf16, isOutput=False)

    stage_d = nc.declare_dram_parameter("stage", [2, NT + 1, D], bf16, isOutput=True)
    out_d = nc.declare_dram_parameter("out", [NT, D], f32, isOutput=True)
    gsum_d = nc.declare_dram_parameter("gsum", [1, E], f32, isOutput=True)
    lst_bounce = nc.dram_tensor("lst_bounce", [16, 2 * 2 * E * FW], i16)

    with tile.TileContext(nc) as tc:
        import contextlib
        with contextlib.ExitStack() as ctx:
            sb = ctx.enter_context(tc.tile_pool(name="sb", bufs=1))
            wk = ctx.enter_context(tc.tile_pool(name="wk", bufs=3))
            big = ctx.enter_context(tc.tile_pool(name="big", bufs=2))
            ps = ctx.enter_context(tc.tile_pool(name="ps", bufs=2, space="PSUM"))
            psg = ctx.enter_context(tc.tile_pool(name="psg", bufs=1, space="PSUM"))
            pse = ctx.enter_context(tc.tile_pool(name="pse", bufs=3, space="PSUM"))

            # ---- loads ----
            xhi_n = sb.tile([128, G, J], bf16, tag="xhi_n")
            nc.gpsimd.dma_start(xhi_n[:], xhi_d.rearrange("(r p) d -> p r d", p=128))
            xlo_n = sb.tile([128, G, J], bf16, tag="xlo_n")
            nc.gpsimd.dma_start(xlo_n[:], xlo_d.rearrange("(r p) d -> p r d", p=128))
            noi_n = sb.tile([128, G, E], f32, tag="noi_n")
            nc.gpsimd.dma_start(noi_n[:], noi_d.rearrange("(r p) e -> p r e", p=128))
            gwT = sb.tile([128, 2, E], f32, tag="gwT")
            nc.gpsimd.dma_start(gwT[:], gwT_d.rearrange("(b p) e -> p b e", p=128))
            wT = sb.tile([128, E, 2, D], bf16, tag="wT")
            nc.gpsimd.dma_start(wT[:], wT_d.rearrange("e b p d -> p e b d"))
            i128 = sb.tile([128, 128], f32, tag="i128")
            nc.gpsimd.dma_start(i128[:], i128_d[:, :])
            em64 = sb.tile([128, E], f32, tag="em64")
            nc.gpsimd.dma_start(em64[:], em64_d[:, :])
            iow = sb.tile([16, G, 8], f32, tag="iow")
            nc.gpsimd.dma_start(iow[:], iow_d[:, :, :])
            ioid = sb.tile([128, NT // 16], i16, tag="ioid")
            nc.gpsimd.dma_start(ioid[:], ioid_d[:, :])
            if has_gate_b:
                gb = sb.tile([1, E], f32, tag="gb")
                nc.gpsimd.dma_start(gb[:], gb_d[:, :])
                ones1 = sb.tile([1, 128], f32, tag="ones1")
                nc.vector.memset(ones1[:], 1.0)
            if has_eb:
                ebt = sb.tile([1, E, D], bf16, tag="ebt")
                nc.gpsimd.dma_start(ebt[:], bass.AP(eb_d, 0, [[E * D, 1], [D, E], [1, D]]))
                ones1b = sb.tile([1, 128], bf16, tag="ones1b")
                nc.vector.memset(ones1b[:], 1.0)
            ones128 = sb.tile([128, 1], f32, tag="ones128")
            nc.vector.memset(ones128[:], 1.0)

            # ---- transposed x via identity dma_gather ----
            CH = 512
            NCH = NT // CH
            xThi = big.tile([128, NCH, 2, CH], bf16, tag="bigb", name="xThi")
            xTlo = big.tile([128, NCH, 2, CH], bf16, tag="bigb", name="xTlo")
            for src, dst in ((xhi_n, xThi), (xlo_n, xTlo)):
                for k in range(NCH):
                    nc.gpsimd.dma_gather(
                        dst[:, k, :, :], src[:],
                        ioid[:, k * (CH // 16):(k + 1) * (CH // 16)],
                        num_idxs=CH, num_idxs_reg=CH, elem_size=J, transpose=True,
                        sbuf_tokens_per_rank=128, sbuf_free_dim_per_rank=J * 2,
                    )
            xT32 = sb.tile([128, NCH, 2, CH], f32, tag="xT32")
            nc.vector.tensor_tensor(xT32[:], xThi[:], xTlo[:], op=mybir.AluOpType.add)

            # ---- gate matmul (true fp32) ----
            logits = sb.tile([128, G, E], f32, tag="logits")
            for g in range(G):
                pg = ps.tile([128, E], f32, tag="pg")
                nc.tensor.matmul(pg[:], xT32[:, g // 4, 0, (g % 4) * 128:(g % 4 + 1) * 128], gwT[:, 0, :], start=True, stop=False)
                nc.tensor.matmul(pg[:], xT32[:, g // 4, 1, (g % 4) * 128:(g % 4 + 1) * 128], gwT[:, 1, :], start=False, stop=not has_gate_b)
                if has_gate_b:
                    nc.tensor.matmul(pg[:], ones1[:, :], gb[:, :], start=False, stop=True)
                nc.scalar.activation(logits[:, g, :], pg[:], AF.Copy)
            nc.vector.tensor_scalar_mul(noi_n[:], noi_n[:], NOISE_STD)
            nc.vector.tensor_tensor(logits[:], logits[:], noi_n[:], op=OP.add)

            # ---- top-2 on logits ----
            m1 = sb.tile([128, G], f32, tag="m1")
            nc.vector.tensor_reduce(m1[:], logits[:], axis=AX.X, op=OP.max)
            eq1 = sb.tile([128, G, E], f32, tag="eq1")
            a_ = m1[:]; m1b = bass.AP(a_.tensor, a_.offset, [a_.ap[0], [1, G], [0, E]])
            nc.vector.tensor_tensor(eq1[:], logits[:], m1b, op=OP.is_equal)
            a_ = em64[:]; emb = bass.AP(a_.tensor, a_.offset, [a_.ap[0], [0, G], [1, E]])
            v1 = sb.tile([128, G, E], f32, tag="vv", name="v1")
            nc.vector.tensor_tensor(v1[:], eq1[:], emb, op=OP.mult)
            m1v = sb.tile([128, G], f32, tag="m1v")
            nc.vector.tensor_reduce(m1v[:], v1[:], axis=AX.X, op=OP.min)
            l2 = sb.tile([128, G, E], f32, tag="l2")
            nc.vector.tensor_scalar_mul(l2[:], eq1[:], BIG)
            nc.vector.tensor_tensor(l2[:], logits[:], l2[:], op=OP.subtract)
            m2 = sb.tile([128, G], f32, tag="m2")
            nc.vector.tensor_reduce(m2[:], l2[:], axis=AX.X, op=OP.max)
            eq2 = sb.tile([128, G, E], f32, tag="eq1", name="eq2")
            a_ = m2[:]; m2b = bass.AP(a_.tensor, a_.offset, [a_.ap[0], [1, G], [0, E]])
            nc.vector.tensor_tensor(eq2[:], l2[:], m2b, op=OP.is_equal)
            v2 = sb.tile([128, G, E], f32, tag="vv", name="v2")
            nc.vector.tensor_tensor(v2[:], eq2[:], emb, op=OP.mult)
            m2v = sb.tile([128, G], f32, tag="m2v")
            nc.vector.tensor_reduce(m2v[:], v2[:], axis=AX.X, op=OP.min)

            # ---- softmax pieces ----
            Ex = sb.tile([128, G, E], f32, tag="eq1", name="Ex")
            nc.scalar.activation(Ex[:], logits[:], AF.Exp)
            den = sb.tile([128, G], f32, tag="den")
            nc.vector.tensor_reduce(den[:], Ex[:], axis=AX.X, op=OP.add)
            rec = sb.tile([128, G], f32, tag="rec")
            nc.vector.reciprocal(rec[:], den[:])
            e1 = sb.tile([128, G], f32, tag="e1")
            nc.scalar.activation(e1[:], m1[:], AF.Exp)
            e2 = sb.tile([128, G], f32, tag="e2")
            nc.scalar.activation(e2[:], m2[:], AF.Exp)
            p1 = sb.tile([128, G], f32, tag="p1")
            nc.vector.tensor_tensor(p1[:], e1[:], rec[:], op=OP.mult)
            p2 = sb.tile([128, G], f32, tag="p2")
            nc.vector.tensor_tensor(p2[:], e2[:], rec[:], op=OP.mult)
            Pn = sb.tile([128, G, E], f32, tag="vv", name="Pn")
            a_ = rec[:]; recb = bass.AP(a_.tensor, a_.offset, [a_.ap[0], [1, G], [0, E]])
            nc.vector.tensor_tensor(Pn[:], Ex[:], recb, op=OP.mult)
            # column sums for aux loss
            pgs = psg.tile([1, E], f32, tag="pgs")
            for g in range(G):
                nc.tensor.matmul(pgs[:], ones128[:, :], Pn[:, g, :], start=(g == 0), stop=(g == G - 1))
            gsum_sb = sb.tile([1, E], f32, tag="gsum_sb")
            nc.scalar.activation(gsum_sb[:], pgs[:], AF.Copy)
            nc.gpsimd.dma_start(gsum_d[:, :], gsum_sb[:])

            # ---- probability-scaled x (natural layout), bf16 ----
            xsA = big.tile([128, G, J], bf16, tag="bigb", name="xsA")
            xsB = big.tile([128, G, J], bf16, tag="bigb", name="xsB")
            for g in range(G):
                x32t = wk.tile([128, J], f32, tag="x32t")
                nc.vector.tensor_tensor(x32t[:], xhi_n[:, g, :], xlo_n[:, g, :], op=OP.add)
                nc.vector.tensor_scalar_mul(xsA[:, g, :], x32t[:], p1[:, g:g + 1])
                nc.scalar.activation(xsB[:, g, :], x32t[:], AF.Copy, scale=p2[:, g:g + 1])

            # ---- wrapped idx-value arrays W1/W2 via selector matmuls ----
            Ws = []
            for r, mv in ((0, m1v), (1, m2v)):
                W = sb.tile([16, G, 8], f32, tag=f"W{r}", name=f"W{r}")
                for b in range(8):
                    pw = ps.tile([16, G], f32, tag="pw")
                    nc.tensor.matmul(pw[:], i128[:, 16 * b:16 * (b + 1)], mv[:], start=True, stop=True)
                    nc.vector.tensor_copy(W[:, :, b], pw[:])
                Ws.append(W)

            # ---- masks + sparse_gather lists ----
            lists = sb.tile([16, 2, E, FW], f32, tag="lists")
            nfd = sb.tile([1, 2 * E], u32, tag="nfd")
            mask = sb.tile([16, E // 2, G, 8], f32, tag="mask")
            for r in range(2):
                W = Ws[r]
                for h in range(2):
                    a_ = W[:]; Wb = bass.AP(a_.tensor, a_.offset, [a_.ap[0], [0, E // 2], [1, G * 8]])
                    a_ = em64[0:16, h * (E // 2):(h + 1) * (E // 2)]; emw = bass.AP(a_.tensor, a_.offset, [a_.ap[0], a_.ap[1], [0, G * 8]])
                    nc.vector.tensor_tensor(mask[:], Wb, emw, op=OP.is_equal)
                    a_ = iow[:]; iob = bass.AP(a_.tensor, a_.offset, [a_.ap[0], [0, E // 2], [1, G * 8]])
                    nc.vector.tensor_tensor(mask[:], mask[:], iob, op=OP.mult)
                    nc.scalar.activation(mask[:], mask[:], AF.Copy, bias=-1.0)
                    for e2 in range(E // 2):
                        e = h * (E // 2) + e2
                        nc.gpsimd.sparse_gather(
                            lists[:, r, e, :], mask[:, e2, :, :],
                            num_found=nfd[:, r * E + e:r * E + e + 1],
                        )

            # ---- list fixups: gather idx (pads->0), scatter idx (pads->trash row) ----
            neg = sb.tile([16, 2, E, FW], f32, tag="neg")
            nc.vector.tensor_scalar(neg[:], lists[:], 0.0, float(NT + 1), op0=OP.is_lt, op1=OP.mult)
            nc.vector.tensor_tensor(neg[:], lists[:], neg[:], op=OP.add)
            ls16 = sb.tile([16, 2, E, FW], i16, tag="ls16")
            nc.vector.tensor_copy(ls16[:], neg[:])
            nc.vector.tensor_scalar_max(lists[:], lists[:], 0.0)
            lg16 = sb.tile([16, 2, E, FW], i16, tag="lg16")
            nc.vector.tensor_copy(lg16[:], lists[:])

            # replicate via DRAM bounce to all 128 partitions
            HALF = 2 * E * FW
            nc.gpsimd.dma_start(bass.AP(lst_bounce, 0, [[2 * HALF, 16], [1, HALF]]), lg16[:])
            nc.gpsimd.dma_start(bass.AP(lst_bounce, HALF, [[2 * HALF, 16], [1, HALF]]), ls16[:])
            idxg = sb.tile([128, 2, E, FW], i16, tag="idxg")
            idxs = sb.tile([128, 2, E, FW], i16, tag="idxs")
            for k in range(8):
                nc.gpsimd.dma_start(idxg[16 * k:16 * (k + 1), :, :, :], bass.AP(lst_bounce, 0, [[2 * HALF, 16], [1, HALF]]))
                nc.gpsimd.dma_start(idxs[16 * k:16 * (k + 1), :, :, :], bass.AP(lst_bounce, HALF, [[2 * HALF, 16], [1, HALF]]))

            # ---- expert compute + scatter to staging planes ----
            for r in range(2):
                xs = xsA if r == 0 else xsB
                for e in range(E):
                    xgT = wk.tile([128, 2, C], bf16, tag="xgT")
                    nc.gpsimd.dma_gather(
                        xgT[:], xs[:], idxg[:, r, e, :],
                        num_idxs=C, num_idxs_reg=C, elem_size=J, transpose=True,
                        sbuf_tokens_per_rank=128, sbuf_free_dim_per_rank=J * 2,
                    )
                    y = wk.tile([128, C // 128, D], bf16, tag="y")
                    for g in range(C // 128):
                        pe_ = pse.tile([128, D], f32, tag="pe_")
                        nc.tensor.matmul(pe_[:], xgT[:, 0, g * 128:(g + 1) * 128], wT[:, e, 0, :], start=True, stop=False)
                        nc.tensor.matmul(pe_[:], xgT[:, 1, g * 128:(g + 1) * 128], wT[:, e, 1, :], start=False, stop=not has_eb)
                        if has_eb:
                            nc.tensor.matmul(pe_[:], ones1b[:, :], ebt[:, e, :], start=False, stop=True)
                        if (e + g) % 2 == 0:
                            nc.vector.tensor_copy(y[:, g, :], pe_[:])
                        else:
                            nc.scalar.activation(y[:, g, :], pe_[:], AF.Copy)
                    nc.gpsimd.dma_scatter_add(
                        stage_d[r, :, :], y[:], idxs[:, r, e, :],
                        num_idxs=C, num_idxs_reg=C, elem_size=D,
                    )

            # ---- final combine: out = stageA + stageB (fp32) ----
            for g in range(G):
                sA = wk.tile([128, D], bf16, tag="sA")
                nc.gpsimd.dma_start(sA[:], stage_d[0, g * 128:(g + 1) * 128, :])
                sB = wk.tile([128, D], bf16, tag="sB")
                nc.gpsimd.dma_start(sB[:], stage_d[1, g * 128:(g + 1) * 128, :])
                ob = wk.tile([128, D], f32, tag="ob")
                nc.vector.tensor_tensor(ob[:], sA[:], sB[:], op=OP.add)
                nc.gpsimd.dma_start(out_d[g * 128:(g + 1) * 128, :], ob[:])
    return nc


def _get_runner(has_gate_b, has_eb):
    key = (has_gate_b, has_eb)
    if key not in _CACHE:
        from tile_fix_inline import split_multi_waits
        from bass_runner_inline import SpmdRunner
        nc = _build(has_gate_b, has_eb)
        nc.compile()
        split_multi_waits(nc, max_waits=1)
        _CACHE[key] = SpmdRunner(nc, NCORE)
    return _CACHE[key]


def _consts():
    i128 = np.eye(128, dtype=np.float32)
    em64 = np.tile((np.arange(E, dtype=np.float32) - 64.0)[None, :], (128, 1))
    # iow[q, a, b] = token id + 1 = 128a + 16b + q + 1
    q = np.arange(16)[:, None, None]
    a = np.arange(G)[None, :, None]
    b = np.arange(8)[None, None, :]
    iow = (128 * a + 16 * b + q + 1).astype(np.float32)
    # wrapped identity: position i at (i%16, i//16), replicated to 128 partitions
    ioid = np.zeros((16, NT // 16), np.int16)
    ii = np.arange(NT)
    ioid[ii % 16, ii // 16] = ii
    ioid = np.tile(ioid, (8, 1))
    return i128, em64, iow, ioid


def kernel(x, noise, gate_w, gate_b, experts_w, experts_b):
    x = np.asarray(x, dtype=np.float32)
    noise = np.asarray(noise, dtype=np.float32)
    gate_w = np.asarray(gate_w, dtype=np.float32)
    gate_b = np.asarray(gate_b, dtype=np.float32)
    experts_w = np.asarray(experts_w, dtype=np.float32)
    experts_b = np.asarray(experts_b, dtype=np.float32)
    N = x.shape[0]
    has_gate_b = bool(np.any(gate_b))
    has_eb = bool(np.any(experts_b))
    r = _get_runner(has_gate_b, has_eb)

    xhi = x.astype(ml_dtypes.bfloat16)
    xlo = (x - xhi.astype(np.float32)).astype(ml_dtypes.bfloat16)
    gwT = np.ascontiguousarray(gate_w.T)
    wT = np.ascontiguousarray(
        experts_w.reshape(E, D, 2, 128).transpose(0, 2, 3, 1)
    ).astype(ml_dtypes.bfloat16)
    i128, em64, iow, ioid = _consts()

    in_maps = []
    for c in range(NCORE):
        sl = slice(c * NT, (c + 1) * NT)
        m = {
            "xhi": xhi[sl], "xlo": xlo[sl], "noi": noise[sl],
            "gwT": gwT, "wT": wT, "i128": i128, "em64": em64,
            "iow": iow, "ioid": ioid,
        }
        if has_gate_b:
            m["gb"] = gate_b[None, :]
        if has_eb:
            m["eb"] = experts_b.reshape(E, D).astype(ml_dtypes.bfloat16)
        in_maps.append(m)

    dev = r.device_inputs(in_maps)
    outs = r.run(dev)
    res = r.outputs_to_host(outs)

    out = np.concatenate([res[c]["out"] for c in range(NCORE)], axis=0)
    gs = np.sum([res[c]["gsum"][0] for c in range(NCORE)], axis=0)
    m = gs / np.float32(N)
    loss = np.float32(np.mean((m - 1.0 / E) ** 2) * E)
    return out, loss


# ---- inlined helpers (kernel.py must be self-contained) ----
import sys as _sys
import types as _types


def _make_tile_fix():
    mod = _types.ModuleType("tile_fix_inline")
    import concourse.mybir as mybir

    _ctr = [0]

    def split_multi_waits(nc, max_waits=1):
        for fn in nc.m.functions:
            for bb in fn.blocks:
                new_list = []
                for inst in bb.instructions:
                    si = inst.sync_info
                    if si is not None and si.on_wait and len(si.on_wait) > max_waits:
                        excess = si.on_wait[:-max_waits]
                        keep = si.on_wait[-max_waits:]
                        for w in excess:
                            _ctr[0] += 1
                            ev = mybir.InstEventSemaphore(
                                name=f"I-waitsplit-{_ctr[0]}",
                                engine=inst.engine,
                                sync_info=mybir.SyncInfo(on_wait=[w], on_update=[]),
                                ins=[], outs=[],
                            )
                            new_list.append(ev)
                        inst.sync_info = mybir.SyncInfo(on_wait=keep, on_update=si.on_update)
                    new_list.append(inst)
                bb.instructions[:] = new_list
        return nc

    mod.split_multi_waits = split_multi_waits
    return mod


def _make_runner_mod():
    mod = _types.ModuleType("bass_runner_inline")
    import jax
    from jax.sharding import Mesh, PartitionSpec
    from jax.experimental.shard_map import shard_map
    import concourse.mybir as mybir
    from concourse.bass2jax import _bass_exec_p, install_neuronx_cc_hook, partition_id_tensor

    class SpmdRunner:
        def __init__(self, nc, n_cores=8):
            install_neuronx_cc_hook()
            self.nc = nc
            self.n_cores = n_cores
            partition_name = nc.partition_id_tensor.name if nc.partition_id_tensor else None
            in_names, out_names, out_avals, zero_outs = [], [], [], []
            for alloc in nc.m.functions[0].allocations:
                if not isinstance(alloc, mybir.MemoryLocationSet):
                    continue
                name = alloc.memorylocations[0].name
                if alloc.kind == "ExternalInput":
                    if name != partition_name:
                        in_names.append(name)
                elif alloc.kind == "ExternalOutput":
                    out_avals.append(jax.core.ShapedArray(alloc.tensor_shape, mybir.dt.np(alloc.dtype)))
                    out_names.append(name)
                    zero_outs.append(np.zeros(alloc.tensor_shape, mybir.dt.np(alloc.dtype)))
            self.in_names = list(in_names)
            self.out_names = list(out_names)
            self.out_avals = out_avals
            self.zero_outs = zero_outs
            n_params = len(in_names)
            n_outs = len(out_names)
            all_in_names = list(in_names) + list(out_names)
            if partition_name is not None:
                all_in_names.append(partition_name)

            def _body(*args):
                operands = list(args)
                if partition_name is not None:
                    operands.append(partition_id_tensor())
                outs = _bass_exec_p.bind(
                    *operands,
                    out_avals=tuple(out_avals),
                    in_names=tuple(all_in_names),
                    out_names=tuple(out_names),
                    lowering_input_output_aliases=(),
                    sim_require_finite=True,
                    sim_require_nnan=True,
                    nc=nc,
                )
                return tuple(outs)

            devices = jax.devices()[:n_cores]
            self.mesh = Mesh(np.asarray(devices), ("core",))
            in_specs = (PartitionSpec("core"),) * (n_params + n_outs)
            out_specs = (PartitionSpec("core"),) * n_outs
            self.fn = jax.jit(
                shard_map(_body, mesh=self.mesh, in_specs=in_specs,
                          out_specs=out_specs, check_rep=False),
                keep_unused=True,
            )
            self._sharding = jax.sharding.NamedSharding(self.mesh, PartitionSpec("core"))

        def device_inputs(self, in_maps):
            import jax
            n = self.n_cores
            concat = [
                np.concatenate([np.asarray(in_maps[c][name]) for c in range(n)], axis=0)
                for name in self.in_names
            ]
            concat += [np.zeros((n * z.shape[0], *z.shape[1:]), z.dtype) for z in self.zero_outs]
            return [jax.device_put(a, self._sharding) for a in concat]

        def run(self, dev_args):
            import jax
            outs = self.fn(*dev_args)
            jax.block_until_ready(outs)
            return outs

        def outputs_to_host(self, outs):
            res = []
            for c in range(self.n_cores):
                d = {}
                for i, name in enumerate(self.out_names):
                    full = np.asarray(outs[i])
                    per = full.reshape(self.n_cores, *self.out_avals[i].shape)
                    d[name] = per[c]
                res.append(d)
            return res

    mod.SpmdRunner = SpmdRunner
    return mod


if "tile_fix_inline" not in _sys.modules:
    _sys.modules["tile_fix_inline"] = _make_tile_fix()
if "bass_runner_inline" not in _sys.modules:
    _sys.modules["bass_runner_inline"] = _make_runner_mod()
